# revision 23
# baseline (speedup 1.0000x reference)
"""Patch TileContext._drain_and_barrier: this container's walrus codegen
rejects >2 sem waits on one CTRL (Drain) instruction. Split the kernel-tail
drain's waits across separate nop instructions (1 wait each)."""
import concourse.tile as tile  # noqa
import concourse.mybir as mybir
from concourse.vector_clock import ScopedClock
from concourse._compat import not_none as nn


def _drain_and_barrier_split(self, tick_clock, wait_clock):
    nc = self.nc
    carrier = nc.sync.nop()
    wait_clock.add_sem_waits(carrier.ins, ScopedClock({None: tick_clock.global_clock}))
    si = carrier.ins.sync_info
    waits = list(si.on_wait) if si and si.on_wait else []
    if len(waits) > 1:
        si.on_wait.clear()
        si.on_wait.append(waits[0])
        for w in waits[1:]:
            n2 = nc.sync.nop()
            n2.ins.sync_info = mybir.SyncInfo(on_wait=[w], on_update=[])
    nc.sync.drain()

    nc.all_engine_barrier()
    assert self.sems is not None
    popped = nc._tile_sem_poison_stack.pop()
    assert popped is self._sem_poison
    nc.clear_and_free_semaphores(list(self.sems.allocated().values()))
    nc.all_engine_barrier()


tile.TileContext._drain_and_barrier = _drain_and_barrier_split


# ---- global wait-splitting pass ----
# This walrus build packs at most MAX_WAITS sem-waits per instruction
# (ISA EVENTS struct holds one; codegen can prepend a limited number of
# sync-wait commands). Move excess waits onto InstNoOp carriers.
MAX_WAITS = 2

def fix_waits(nc, max_waits=MAX_WAITS):
    import concourse.mybir as mybir
    dma2 = getattr(nc, "_fix_dma_waits2", False)
    n_fixed = 0
    for fn in nc.m.functions:
        for blk in fn.blocks:
            insts = blk.instructions
            out = []
            for inst in insts:
                lim = max_waits
                if dma2 and isinstance(inst, mybir.InstDMACopy):
                    lim = 2
                si = getattr(inst, "sync_info", None)
                if si is not None and si.on_wait and len(si.on_wait) > lim:
                    waits = list(si.on_wait)
                    si.on_wait.clear()
                    for w in waits[:-lim] if lim else waits:
                        n_fixed += 1
                        nop = mybir.InstNoOp(
                            name=f"{inst.name}.wsplit{n_fixed}",
                            sync_info=mybir.SyncInfo(on_wait=[w], on_update=[]),
                            bass_nofuse=True,
                            engine=inst.engine,
                        )
                        out.append(nop)
                    for w in waits[-lim:] if lim else []:
                        si.on_wait.append(w)
                elif si is not None and si.on_wait and len(si.on_wait) > 1 and getattr(inst, "opcode", None) is None:
                    pass
                out.append(inst)
            blk.instructions = out
    return n_fixed


# auto-apply fix_waits on serialization
import concourse.bass as _bass
_orig_to_json_bytes = _bass.Bass.to_json_bytes

def _to_json_bytes_fixed(self, *a, **kw):
    try:
        fix_waits(self, max_waits=getattr(self, "_fix_max_waits", 1))
    except Exception as e:
        import traceback; traceback.print_exc()
    return _orig_to_json_bytes(self, *a, **kw)

_bass.Bass.to_json_bytes = _to_json_bytes_fixed


"""NodeModel GNN kernel for Trainium2 (Bass/Tile), 8-core SPMD. v2.

Strategy (v2 — fp16 compute, lean device program):
- Shard destination NODES into 8 contiguous ranges of 6250; each core handles
  exactly the edges targeting its nodes (no collectives needed).
- Shared degree-sorted column schedule (same as v1): columns = destination
  nodes grouped by degree descending; round r feeds the r-th edge of each
  still-active column. Segment SUM accumulates in PSUM across rounds via
  matmul; MAX/MIN are running DVE ops; COUNT is host-known (rdeg).
- fp16 everywhere on the device data path (x gathered in fp16, weights fp16,
  activations fp16); PSUM accumulation stays fp32.
- Column-side x (xcolT) and 1/deg are host-prepared per core in schedule
  order, loaded by direct DMA — no column gathers or transposes on device.
- Output is written column-major ([192, cols]) by direct DMA; the host
  un-permutes columns to node order and assembles the final concat (x and
  u[batch] passthrough fields are host-assembled).
- Only per-edge source-row gathers use indirect DMA (128 rows/instruction,
  the Pool-engine SWDGE serial floor dominates the kernel).
"""

import numpy as np

import concourse.bass as bass
import concourse.tile as tile
from concourse.bass import IndirectOffsetOnAxis

F32 = mybir.dt.float32
F16 = mybir.dt.float16
I32 = mybir.dt.int32
AF = mybir.ActivationFunctionType
ALU = mybir.AluOpType

P = 128
W = 512  # tile width (columns = destination nodes)


def build_schedule(col, n_nodes, n_cores):
    """Host-side index preprocessing. Returns shared schedule + per-core arrays."""
    ncore_nodes = n_nodes // n_cores
    deg_all = np.bincount(col, minlength=n_nodes)
    dmax = int(deg_all.max())

    # per-core degree histograms of own nodes
    hist = np.zeros((n_cores, dmax + 1), np.int64)
    for c in range(n_cores):
        d = deg_all[c * ncore_nodes : (c + 1) * ncore_nodes]
        hist[c] = np.bincount(d, minlength=dmax + 1)
    H = hist.max(axis=0)  # shared histogram (per exact degree), index 0 unused

    # shared column degree sequence, descending
    col_degs = np.repeat(np.arange(dmax, 0, -1), H[dmax:0:-1])
    n_cols = len(col_degs)
    n_tiles = (n_cols + W - 1) // W

    # CSR of edges by destination (stable order)
    order = np.argsort(col, kind="stable")
    starts = np.zeros(n_nodes + 1, np.int64)
    np.cumsum(deg_all, out=starts[1:])

    # per-core: map shared columns -> node ids (real) or -1 (virtual)
    col_node = np.full((n_cores, n_cols), -1, np.int64)
    for c in range(n_cores):
        d_own = deg_all[c * ncore_nodes : (c + 1) * ncore_nodes]
        nodes_by_deg = {}
        for ln in np.argsort(-d_own, kind="stable"):
            if d_own[ln] == 0:
                break
            nodes_by_deg.setdefault(int(d_own[ln]), []).append(ln)
        used = {d: 0 for d in range(1, dmax + 1)}
        for j in range(n_cols):
            d = int(col_degs[j])
            lst = nodes_by_deg.get(d, [])
            k = used[d]
            if k < len(lst):
                col_node[c, j] = c * ncore_nodes + lst[k]
                used[d] = k + 1

    # schedule: per tile, list of round widths; global column -> padded pos
    tiles = []
    col_pos = np.zeros(n_cols, np.int64)
    cc = 0
    for t in range(n_tiles):
        j0, j1 = t * W, min((t + 1) * W, n_cols)
        degs = col_degs[j0:j1]
        d_t = int(degs[0])
        widths = [int(np.searchsorted(-degs, -(r + 1), side="right")) for r in range(d_t)]
        tiles.append((j0, j1, widths, cc))
        col_pos[j0:j1] = cc * P + np.arange(j1 - j0)
        cc += (j1 - j0 + P - 1) // P

    n_chunk_slots = sum(sum((w + P - 1) // P for w in widths) for _, _, widths, _ in tiles)
    n_col_chunks = cc

    return dict(
        ncore_nodes=ncore_nodes, deg_all=deg_all, col_degs=col_degs,
        n_cols=n_cols, n_tiles=n_tiles, tiles=tiles, order=order, starts=starts,
        col_node=col_node, col_pos=col_pos,
        n_chunk_slots=n_chunk_slots, n_col_chunks=n_col_chunks, dmax=dmax,
    )


def iter_chunks(sched):
    """Yield the shared chunk-slot structure: ('pair', ti, r, b) covers rounds
    (r, r+1) chunk b; ('single', ti, r, b) covers round r chunk b alone.
    Pair chunks exist for even r where chunk b also exists in round r+1."""
    for ti, (j0, j1, widths, cc0) in enumerate(sched["tiles"]):
        d_t = len(widths)
        for r in range(0, d_t, 2):
            w_r = widths[r]
            w_n = widths[r + 1] if r + 1 < d_t else 0
            nrk_r = (w_r + P - 1) // P
            nrk_n = (w_n + P - 1) // P
            for b in range(nrk_n):
                yield ("pair", ti, r, b)
            for b in range(nrk_n, nrk_r):
                yield ("single", ti, r, b)


def build_pair_layout(sched, row, n_cores):
    """Per-core Eulerian pairing: returns x_perm row lists and chunk offset
    arrays. For 'pair' chunks, offset q reads x_perm rows (q, q+1) = the two
    sources of that column's rounds (r, r+1) (edge order within a column is
    chosen to match the trail orientation). For 'single' chunks, offset reads
    one row. Sources per column live in srcs[column] in round order."""
    order, starts = sched["order"], sched["starts"]
    col_node = sched["col_node"]
    deg_all = sched["deg_all"]
    tiles = sched["tiles"]

    chunks = list(iter_chunks(sched))
    n_pair = sum(1 for c in chunks if c[0] == "pair")
    n_single = sum(1 for c in chunks if c[0] == "single")

    idx_pair = np.zeros((n_cores, P, max(n_pair, 1)), np.int32)
    idx_single = np.zeros((n_cores, P, max(n_single, 1)), np.int32)
    perm_rows = []
    for c in range(n_cores):
        nodes_all = col_node[c]
        # per-column source lists in (initial) round order
        srcs = {}
        for j in range(sched["n_cols"]):
            n = nodes_all[j]
            if n >= 0:
                d = int(deg_all[n])
                srcs[j] = [int(row[order[starts[n] + r]]) for r in range(d)]

        # demand edges: per column, consecutive pairs (round 2k, 2k+1)
        demands = []  # (u, v, j, r) — column j rounds (r, r+1)
        for j, s in srcs.items():
            d = len(s)
            for r in range(0, d - 1, 2):
                demands.append([s[r], s[r + 1], j, r])

        # Eulerian trails over the demand multigraph
        from collections import defaultdict
        adj = defaultdict(list)  # u -> list of demand indices
        for di, (u, v, j, r) in enumerate(demands):
            adj[u].append(di)
            if v != u:
                adj[v].append(di)
        used = [False] * len(demands)
        pos_of = {}   # demand idx -> (q, oriented_u_first: bool)
        seq = []      # x_perm row ids
        deg_left = {u: len(l) for u, l in adj.items()}
        # stack-based Hierholzer from every odd / remaining vertex
        def consume(u):
            """Walk a trail from u, appending rows to seq."""
            stack = [u]
            trail = [u]
            while stack:
                v = stack[-1]
                found = None
                while adj[v]:
                    di = adj[v].pop()
                    if not used[di]:
                        found = di
                        break
                if found is None:
                    stack.pop()
                    if stack:
                        trail.append(stack[-1])
                    continue
                used[found] = True
                uu, vv, _, _ = demands[found]
                nxt = vv if uu == v else uu
                stack.append(nxt)
                trail.append(nxt)
            return trail

        # Hierholzer with trail splicing is complex; use simple edge-walk:
        # repeatedly start at a vertex with unused edges and walk greedily.
        # Each walk is appended to seq; demand positions recorded on the fly.
        for start_u in list(adj.keys()):
            while adj[start_u] and not all(used[di] for di in adj[start_u]):
                u = start_u
                walk = [u]
                while True:
                    di = None
                    while adj[u]:
                        cand = adj[u][-1]
                        if used[cand]:
                            adj[u].pop()
                            continue
                        di = cand
                        adj[u].pop()
                        break
                    if di is None:
                        break
                    used[di] = True
                    uu, vv, _, _ = demands[di]
                    nxt = vv if uu == u else uu
                    q = len(seq) + len(walk) - 1
                    pos_of[di] = (q, uu == u)
                    walk.append(nxt)
                    u = nxt
                if len(walk) > 1:
                    seq.extend(walk)
                else:
                    break

        # rows needed by singles or unpaired uses but absent from seq
        first_pos = {}
        for q, rid in enumerate(seq):
            if rid not in first_pos:
                first_pos[rid] = q
        extra = []
        for j, s in srcs.items():
            for rid in s:
                if rid not in first_pos:
                    first_pos[rid] = len(seq) + len(extra)
                    extra.append(rid)
        seq = seq + extra
        perm_rows.append(np.array(seq + [0], dtype=np.int64))  # +1 guard row

        # apply orientation swaps to srcs (so round r = x_perm[q], r+1 = q+1)
        for di, (u, v, j, r) in enumerate(demands):
            if di in pos_of:
                q, u_first = pos_of[di]
                if not u_first:  # v placed first: swap edges r, r+1
                    srcs[j][r], srcs[j][r + 1] = srcs[j][r + 1], srcs[j][r]
                    demands[di][0], demands[di][1] = v, u

        # fill chunk offset arrays
        pair_pos = {di: pos_of[di][0] for di in pos_of}
        # column+round -> demand idx
        dem_at = {(d[2], d[3]): di for di, d in enumerate(demands)}
        pc = 0
        scn = 0
        for kind, ti, r, b in chunks:
            j0, j1, widths, cc0 = tiles[ti]
            d_t = len(widths)
            w_r = widths[r]
            a0, a1 = b * P, min((b + 1) * P, w_r)
            for li, j in enumerate(range(j0 + a0, j0 + a1)):
                n = nodes_all[j]
                if n < 0 or int(deg_all[n]) <= r:
                    q = 0
                elif kind == "pair" and (j, r) in dem_at and dem_at[(j, r)] in pair_pos:
                    q = pair_pos[dem_at[(j, r)]]
                else:
                    # single slot (odd-degree last round, or width boundary)
                    q = first_pos[srcs[j][r]]
                if kind == "pair":
                    idx_pair[c, li, pc] = q
                else:
                    idx_single[c, li, scn] = q
            if kind == "pair":
                pc += 1
            else:
                scn += 1

    max_m = max(len(p) for p in perm_rows)
    return dict(idx_pair=idx_pair, idx_single=idx_single, perm_rows=perm_rows,
                n_pair=n_pair, n_single=n_single, max_m=max_m, chunks=chunks)


def fill_row_indices(sched, row, n_cores):
    """Per-core gather index array idx_row [P, n_chunk_slots] (source of the
    r-th edge of each column; 0 for virtual/padding lanes)."""
    order, starts = sched["order"], sched["starts"]
    col_node = sched["col_node"]
    tiles = sched["tiles"]
    deg_all = sched["deg_all"]
    idx_row = np.zeros((n_cores, P, sched["n_chunk_slots"]), np.int32)

    for c in range(n_cores):
        sc = 0
        for (j0, j1, widths, _cc0) in tiles:
            nodes = col_node[c, j0:j1]
            for r, w in enumerate(widths):
                for b in range((w + P - 1) // P):
                    a, e = b * P, min((b + 1) * P, w)
                    nb = nodes[a:e]
                    rb = (nb >= 0) & (r < deg_all[np.where(nb >= 0, nb, 0)])
                    src = np.zeros(e - a, np.int64)
                    sel = np.where(rb)[0]
                    if len(sel):
                        eidx = order[starts[nb[sel]] + r]
                        src[sel] = row[eidx]
                    idx_row[c, : e - a, sc] = src
                    sc += 1
    return idx_row


def build_kernel(sched, pl, hid_ch, lat_ch, repeat=1, tune=None):
    """Emit the Bass program (shared across cores). `pl` is the pair layout
    from build_pair_layout. repeat>1 re-runs the whole tile loop (for timing
    regression only; outputs are simply overwritten)."""
    t = dict(gat_bufs=12, gat2_bufs=8, tr_bufs=2, h_bufs=2, h3_bufs=1,
             sum_bufs=1, xrow_bufs=4, act_bufs=4, skip_compute=False,
             fake_gather=False, lookahead=2)
    t.update(tune or {})
    nc = bass.Bass(dynamic_dma_scratch_size=49152)
    tiles = sched["tiles"]
    ncc = sched["n_col_chunks"]
    n_pair, n_single, max_m = pl["n_pair"], pl["n_single"], pl["max_m"]

    # per-(ti, r) chunk-column bases into idx_pair / idx_single
    pair_base, single_base = {}, {}
    pc = scn = 0
    for kind, ti, r, b in pl["chunks"]:
        if kind == "pair":
            pair_base.setdefault((ti, r), pc)
            pc += 1
        else:
            single_base.setdefault((ti, r), scn)
            scn += 1

    xperm_t = nc.dram_tensor("x_perm", [max_m + 2, 64], F16, kind="ExternalInput")
    idxp_t = nc.dram_tensor("idx_pair", [P, max(n_pair, 1)], I32, kind="ExternalInput")
    idxs_t = nc.dram_tensor("idx_single", [P, max(n_single, 1)], I32, kind="ExternalInput")
    xcolT_t = nc.dram_tensor("xcolT", [64, ncc * P], F16, kind="ExternalInput")
    rdeg_t = nc.dram_tensor("rdeg", [64, ncc * P], F32, kind="ExternalInput")
    w1a_t = nc.dram_tensor("W1a", [64, hid_ch], F16, kind="ExternalInput")
    w1b_t = nc.dram_tensor("W1b", [64, hid_ch], F16, kind="ExternalInput")
    w2_t = nc.dram_tensor("W2", [hid_ch, hid_ch], F16, kind="ExternalInput")
    w3_t = nc.dram_tensor("W3", [hid_ch, lat_ch], F16, kind="ExternalInput")
    b1_t = nc.dram_tensor("b1", [hid_ch, 1], F32, kind="ExternalInput")
    b2_t = nc.dram_tensor("b2", [hid_ch, 1], F32, kind="ExternalInput")
    b3_t = nc.dram_tensor("b3", [lat_ch, 1], F32, kind="ExternalInput")
    ident_t = nc.dram_tensor("ident", [P, P], F16, kind="ExternalInput")

    outT_t = nc.dram_tensor("outT", [3 * lat_ch, ncc * P], F32, kind="ExternalOutput")
    tok_t = nc.dram_tensor("tok", [P, 1], F32, kind="ExternalInput")
    tokout_t = nc.dram_tensor("tok_out", [P, 1], F32, kind="ExternalOutput")

    with tile.TileContext(nc) as tc:
        with (
            tc.tile_pool(name="const", bufs=1) as constp,
            tc.tile_pool(name="idxp", bufs=1) as idxp,
            tc.tile_pool(name="gat", bufs=t["gat_bufs"]) as gatp,
            tc.tile_pool(name="xrow", bufs=t["xrow_bufs"]) as xrowp,
            tc.tile_pool(name="xcol", bufs=2) as xcolp,
            tc.tile_pool(name="act", bufs=t["act_bufs"]) as actp,
            tc.tile_pool(name="mm", bufs=4) as mmp,
            tc.tile_pool(name="stage", bufs=8) as stagep,
            tc.tile_pool(name="ps_tr", bufs=t["tr_bufs"], space="PSUM") as ps_tr,
            tc.tile_pool(name="ps_h", bufs=t["h_bufs"], space="PSUM") as ps_h,
            tc.tile_pool(name="ps_h3", bufs=t["h3_bufs"], space="PSUM") as ps_h3,
            tc.tile_pool(name="ps_sum", bufs=t["sum_bufs"], space="PSUM") as ps_sum,
        ):
            # constants
            w1a = constp.tile([64, hid_ch], F16); nc.sync.dma_start(w1a[:], w1a_t[:])
            w1b = constp.tile([64, hid_ch], F16); nc.sync.dma_start(w1b[:], w1b_t[:])
            w2 = constp.tile([hid_ch, hid_ch], F16); nc.sync.dma_start(w2[:], w2_t[:])
            w3 = constp.tile([hid_ch, lat_ch], F16); nc.sync.dma_start(w3[:], w3_t[:])
            b1 = constp.tile([hid_ch, 1], F32); nc.sync.dma_start(b1[:], b1_t[:])
            b2 = constp.tile([hid_ch, 1], F32); nc.sync.dma_start(b2[:], b2_t[:])
            b3 = constp.tile([lat_ch, 1], F32); nc.sync.dma_start(b3[:], b3_t[:])
            ident = constp.tile([P, P], F16); nc.sync.dma_start(ident[:], ident_t[:])
            idx_pair_sb = idxp.tile([P, max(n_pair, 1)], I32)
            nc.sync.dma_start(idx_pair_sb[:], idxp_t[:])
            idx_single_sb = idxp.tile([P, max(n_single, 1)], I32)
            nc.sync.dma_start(idx_single_sb[:], idxs_t[:])
            tok_sb = idxp.tile([P, 1], F32)
            nc.sync.dma_start(tok_sb[:], tok_t[:])
            nc.sync.dma_start(tokout_t[:], tok_sb[:])
            rdeg_sb = idxp.tile([64, ncc * P], F32)
            nc.sync.dma_start(rdeg_sb[:], rdeg_t[:])

            # Flatten (tile, round) into one software-pipelined schedule:
            # stageA (gather + transpose + copy) runs `lookahead` rounds
            # ahead of stageB (matmuls + relus + minmax + finalize).
            flat = []  # (tile_idx, r, w, sc0, nrk)
            sc = 0
            for ti, (j0, j1, widths, cc0) in enumerate(tiles):
                for r, w in enumerate(widths):
                    nrk = (w + P - 1) // P
                    flat.append((ti, r, w, sc, nrk))
                    sc += nrk

            for _rep in range(repeat):
                tstate = {}  # tile_idx -> (xcolT, psum, vmax, vmin)
                arts = {}    # flat idx -> xrowT
                pairbuf = {}  # (ti, r, b) -> g2 tile (even r; second half = r+1)

                def stageA(fi):
                    ti, r, w, sc0, nrk = flat[fi]
                    widths = tiles[ti][2]
                    if r == 0:
                        j0, j1, widths, cc0 = tiles[ti]
                        xcolT = xcolp.tile([64, W], F16, tag="xcolT")
                        nc.sync.dma_start(xcolT[:, : j1 - j0],
                                          xcolT_t[:, cc0 * P : cc0 * P + (j1 - j0)])
                        psum = ps_sum.tile([lat_ch, W], F32, tag="psum")
                        vmax = mmp.tile([lat_ch, W], F16, tag="vmax")
                        vmin = mmp.tile([lat_ch, W], F16, tag="vmin")
                        tstate[ti] = (xcolT, psum, vmax, vmin)
                    srcs = []  # per chunk: (tile, col_slice) views for transpose
                    if r % 2 == 0:
                        w_n = widths[r + 1] if r + 1 < len(widths) else 0
                        nrk_n = (w_n + P - 1) // P
                        pb = pair_base.get((ti, r))
                        for b in range(nrk_n):
                            g2 = gatp.tile([P, 128], F16, tag="g2")
                            if t["fake_gather"]:
                                nc.sync.dma_start(g2[:], xperm_t[0 : 2 * P, :].rearrange("(a b) c -> a (b c)", b=2))
                            else:
                                nc.gpsimd.indirect_dma_start(
                                    out=g2[:], out_offset=None, in_=xperm_t[:],
                                    in_offset=IndirectOffsetOnAxis(
                                        ap=idx_pair_sb[:, pb + b : pb + b + 1], axis=0))
                            pairbuf[(ti, r, b)] = g2
                            srcs.append(g2[:, 0:64])
                        sb = single_base.get((ti, r))
                        for b in range(nrk_n, nrk):
                            g = gatp.tile([P, 64], F16, tag="g")
                            if t["fake_gather"]:
                                nc.sync.dma_start(g[:], xperm_t[0:P, :])
                            else:
                                nc.gpsimd.indirect_dma_start(
                                    out=g[:], out_offset=None, in_=xperm_t[:],
                                    in_offset=IndirectOffsetOnAxis(
                                        ap=idx_single_sb[:, sb + (b - nrk_n) : sb + (b - nrk_n) + 1], axis=0))
                            srcs.append(g[:])
                    else:
                        for b in range(nrk):
                            g2 = pairbuf.pop((ti, r - 1, b))
                            srcs.append(g2[:, 64:128])
                    ptr = ps_tr.tile([64, W], F16, tag="ptr")
                    for b in range(nrk):
                        nc.tensor.transpose(out=ptr[:, b * P : (b + 1) * P],
                                            in_=srcs[b], identity=ident[:])
                    xrowT = xrowp.tile([64, W], F16, tag="xrowT")
                    nc.vector.tensor_copy(xrowT[:, : nrk * P], ptr[:, : nrk * P])
                    arts[fi] = xrowT

                def stageB(fi):
                    ti, r, w, sc0, nrk = flat[fi]
                    j0, j1, widths, cc0 = tiles[ti]
                    wt = j1 - j0
                    d_t = len(widths)
                    xcolT, psum, vmax, vmin = tstate[ti]
                    xrowT = arts.pop(fi)
                    h1p = ps_h.tile([hid_ch, W], F32, tag="h1p")
                    nc.tensor.matmul(out=h1p[:, :w], lhsT=w1a[:], rhs=xrowT[:, :w], start=True, stop=False)
                    nc.tensor.matmul(out=h1p[:, :w], lhsT=w1b[:], rhs=xcolT[:, :w], start=False, stop=True)
                    h1 = actp.tile([hid_ch, W], F16, tag="h1")
                    nc.scalar.activation(h1[:, :w], h1p[:, :w], AF.Relu, bias=b1[:])
                    h2p = ps_h.tile([hid_ch, W], F32, tag="h2p")
                    nc.tensor.matmul(out=h2p[:, :w], lhsT=w2[:], rhs=h1[:, :w], start=True, stop=True)
                    h2 = actp.tile([hid_ch, W], F16, tag="h2")
                    nc.scalar.activation(h2[:, :w], h2p[:, :w], AF.Relu, bias=b2[:])
                    h3p = ps_h3.tile([lat_ch, W], F32, tag="h3p")
                    nc.tensor.matmul(out=h3p[:, :w], lhsT=w3[:], rhs=h2[:, :w], start=True, stop=True)
                    nc.tensor.matmul(out=psum[:, :w], lhsT=w3[:], rhs=h2[:, :w],
                                     start=(r == 0), stop=(r == d_t - 1), skip_group_check=True)
                    if r == 0:
                        nc.vector.tensor_copy(vmax[:, :w], h3p[:, :w])
                        nc.vector.tensor_copy(vmin[:, :w], h3p[:, :w])
                    else:
                        nc.vector.tensor_tensor(out=vmax[:, :w], in0=vmax[:, :w], in1=h3p[:, :w], op=ALU.max)
                        nc.vector.tensor_tensor(out=vmin[:, :w], in0=vmin[:, :w], in1=h3p[:, :w], op=ALU.min)
                    if r == d_t - 1:
                        # finalize tile: mean/max/min + b3, column-major writes
                        mean_s = stagep.tile([lat_ch, W], F32, tag="mean_s")
                        nc.vector.tensor_tensor(out=mean_s[:, :wt], in0=psum[:, :wt],
                                                in1=rdeg_sb[:, cc0 * P : cc0 * P + wt], op=ALU.mult)
                        mean_f = stagep.tile([lat_ch, W], F32, tag="mean_f")
                        nc.scalar.activation(mean_f[:, :wt], mean_s[:, :wt], AF.Identity, bias=b3[:])
                        max_f = stagep.tile([lat_ch, W], F32, tag="max_f")
                        nc.scalar.activation(max_f[:, :wt], vmax[:, :wt], AF.Identity, bias=b3[:])
                        min_f = stagep.tile([lat_ch, W], F32, tag="min_f")
                        nc.scalar.activation(min_f[:, :wt], vmin[:, :wt], AF.Identity, bias=b3[:])
                        nc.sync.dma_start(outT_t[0:lat_ch, cc0 * P : cc0 * P + wt], mean_f[:, :wt])
                        nc.sync.dma_start(outT_t[lat_ch : 2 * lat_ch, cc0 * P : cc0 * P + wt], max_f[:, :wt])
                        nc.sync.dma_start(outT_t[2 * lat_ch : 3 * lat_ch, cc0 * P : cc0 * P + wt], min_f[:, :wt])

                L = t["lookahead"]
                n_flat = len(flat)
                for fi in range(min(L, n_flat)):
                    stageA(fi)
                for fi in range(n_flat):
                    if fi + L < n_flat:
                        stageA(fi + L)
                    if not t["skip_compute"]:
                        stageB(fi)
    return nc


# ---------------- public entry point ----------------

N_NODES = 50000
N_EDGES = 800000
IN_CH = 64
HID_CH = 128
LAT_CH = 64
N_GRAPHS = 64
U_DIM = 32
N_CORES = 8


def make_in_maps(sched, pl, x, W1, W2, W3, b1, b2, b3):
    """Per-core input dicts (shared program, per-core data)."""
    x16 = x.astype(np.float16)
    ncc = sched["n_col_chunks"]
    col_node = sched["col_node"]
    col_pos = sched["col_pos"]
    deg_all = sched["deg_all"]
    ident = np.eye(P, dtype=np.float16)
    max_m = pl["max_m"]

    in_maps = []
    for c in range(N_CORES):
        nodes = col_node[c]  # [n_cols], -1 virtual
        real = nodes >= 0
        xcolT = np.zeros((64, ncc * P), np.float16)
        xcolT[:, col_pos[real]] = x16[nodes[real]].T
        rdeg = np.ones((1, ncc * P), np.float32)
        rdeg[0, col_pos[real]] = 1.0 / deg_all[nodes[real]]
        rdeg = np.broadcast_to(rdeg, (64, ncc * P)).copy()
        x_perm = np.zeros((max_m + 2, 64), np.float16)
        pr = pl["perm_rows"][c]
        x_perm[: len(pr)] = x16[pr]
        in_maps.append({
            "x_perm": x_perm,
            "idx_pair": pl["idx_pair"][c], "idx_single": pl["idx_single"][c],
            "xcolT": xcolT, "rdeg": rdeg,
            "W1a": W1[:64].astype(np.float16), "W1b": W1[64:].astype(np.float16),
            "W2": W2.astype(np.float16), "W3": W3.astype(np.float16),
            "b1": np.ascontiguousarray(b1[:, None].astype(np.float32)),
            "b2": np.ascontiguousarray(b2[:, None].astype(np.float32)),
            "b3": np.ascontiguousarray(b3[:, None].astype(np.float32)),
            "ident": ident,
            "tok": np.zeros((P, 1), np.float32),
        })
    return in_maps


def assemble_output(sched, res_list, x, u, batch):
    """Un-permute per-core column-major results and build the full output."""
    n_nodes = x.shape[0]
    col_node = sched["col_node"]
    col_pos = sched["col_pos"]
    out = np.zeros((n_nodes, 288), np.float32)
    out[:, 0:64] = x
    out[:, 256:288] = u[batch]
    for c in range(N_CORES):
        outT = res_list[c]["outT"]  # [192, ncc*P]
        nodes = col_node[c]
        real = nodes >= 0
        out[nodes[real], 64:256] = outT[:, col_pos[real]].T
    return out


def kernel(**inputs):
    """Full-input NodeModel forward. Returns [N_NODES, 288] float32."""
    from concourse.bass_utils import run_bass_kernel_spmd

    x = np.asarray(inputs["x"], np.float32)
    edge_index = np.asarray(inputs["edge_index"])
    u = np.asarray(inputs["u"], np.float32)
    batch = np.asarray(inputs["batch"])
    W1 = np.asarray(inputs["W1"], np.float32)
    b1 = np.asarray(inputs["b1"], np.float32)
    W2 = np.asarray(inputs["W2"], np.float32)
    b2 = np.asarray(inputs["b2"], np.float32)
    W3 = np.asarray(inputs["W3"], np.float32)
    b3 = np.asarray(inputs["b3"], np.float32)

    row = edge_index[0].astype(np.int32)
    col = edge_index[1].astype(np.int32)

    sched = build_schedule(col, x.shape[0], N_CORES)
    pl = build_pair_layout(sched, row, N_CORES)

    nc = build_kernel(sched, pl, W2.shape[0], W3.shape[1])
    in_maps = make_in_maps(sched, pl, x, W1, W2, W3, b1, b2, b3)

    res = run_bass_kernel_spmd(nc, in_maps, core_ids=list(range(N_CORES)))
    return assemble_output(sched, res.results, x, u, batch).astype(np.float32)
